# revision 6
# baseline (speedup 1.0000x reference)
"""Tensor-parallel attention kernel for 8 Trainium2 NeuronCores.

Shards the 32 attention heads across 8 cores (4 heads each): wq/wk/wv are
sharded column-wise (rows of the weight matrices), wo row-wise; x is
replicated. Each core computes a partial output (its heads' contribution
through wo) and the host sums the 8 partials.

All device matmuls run in float32r (fp32 storage, reduced-precision PE mode,
full bf16-rate for moving dims >= 256), accumulating in f32 PSUM.
"""

import math
import sys

sys.path.insert(0, "/opt/trn_rl_repo")

import numpy as np

import concourse.bacc as bacc
import concourse.bass as bass
import concourse.mybir as mybir
import concourse.tile as tile

F32 = mybir.dt.float32
F32R = mybir.dt.float32r
AF = mybir.ActivationFunctionType
ALU = mybir.AluOpType

HEAD_DIM = 128
NEG = -1.0e30


class Cfg:
    def __init__(self, B=2, S=2048, D=4096, H_PER=4, n_cores=8):
        self.B, self.S, self.D, self.H_PER = B, S, D, H_PER
        self.n_cores = n_cores
        self.T = B * S                    # total tokens (batch-major)
        self.O = H_PER * HEAD_DIM         # per-core projection width
        self.DC = D // 128                # contraction chunks
        self.TT = self.T // 128           # 128-token tiles
        self.TM = self.T // 512           # 512-token macro tiles
        self.NJT = S // 128               # max j-tiles per batch
        self.IMACS = S // 512             # 512-wide i-macros per batch
        self.EC = D // 512                # 512-wide e-chunks of the output


def build(cfg: Cfg) -> bacc.Bacc:
    B, S, D, T, O = cfg.B, cfg.S, cfg.D, cfg.T, cfg.O
    H_PER, DC, TT, TM = cfg.H_PER, cfg.DC, cfg.TT, cfg.TM
    scale = 1.0 / math.sqrt(HEAD_DIM)

    nc = bacc.Bacc(None, target_bir_lowering=False)

    xt = nc.dram_tensor("xt", [D, T], F32R, kind="ExternalInput")
    wqt = nc.dram_tensor("wqt", [D, O], F32R, kind="ExternalInput")
    wkt = nc.dram_tensor("wkt", [D, O], F32R, kind="ExternalInput")
    wvt = nc.dram_tensor("wvt", [D, O], F32R, kind="ExternalInput")
    wot = nc.dram_tensor("wot", [O, D], F32R, kind="ExternalInput")
    cose = nc.dram_tensor("cose", [T, 64], F32, kind="ExternalInput")
    sine = nc.dram_tensor("sine", [T, 64], F32, kind="ExternalInput")
    maskt = nc.dram_tensor("maskt", [4 * 128, 512], F32, kind="ExternalInput")
    ident = nc.dram_tensor("ident", [128, 128], F32R, kind="ExternalInput")
    onesm = nc.dram_tensor("onesm", [128, 128], F32R, kind="ExternalInput")
    out = nc.dram_tensor("out", [T, D], F32, kind="ExternalOutput")

    # DRAM scratch for projected q/k (rotated) and v, [token, O] layout
    q_s = nc.dram_tensor("q_s", [T, O], F32R)
    k_s = nc.dram_tensor("k_s", [T, O], F32R)
    v_s = nc.dram_tensor("v_s", [T, O], F32R)

    with tile.TileContext(nc) as tc:
        # ---- constants used across phases ----
        with tc.tile_pool(name="const", bufs=1) as cpool:
            cos_sb = cpool.tile([128, TT, 64], F32, name="cos_sb")
            sin_sb = cpool.tile([128, TT, 64], F32, name="sin_sb")
            mask_sb = cpool.tile([128, 4, 512], F32, name="mask_sb")
            id_sb = cpool.tile([128, 128], F32R, name="id_sb")
            ones_sb = cpool.tile([128, 128], F32R, name="ones_sb")
            nc.sync.dma_start(out=cos_sb[:], in_=cose[:].rearrange("(tt p) k -> p tt k", p=128))
            nc.sync.dma_start(out=sin_sb[:], in_=sine[:].rearrange("(tt p) k -> p tt k", p=128))
            nc.sync.dma_start(out=mask_sb[:], in_=maskt[:].rearrange("(m p) j -> p m j", p=128))
            nc.sync.dma_start(out=id_sb[:], in_=ident[:])
            nc.sync.dma_start(out=ones_sb[:], in_=onesm[:])

            # ================= Phase 1a: Q,K projections + RoPE =============
            with tc.tile_pool(name="wqk", bufs=1) as wpool, \
                 tc.tile_pool(name="p1sb", bufs=1) as sb, \
                 tc.tile_pool(name="p1ps", bufs=1, space="PSUM") as ps:
                wq_sb = wpool.tile([128, DC, O], F32R, name="wq_sb")
                wk_sb = wpool.tile([128, DC, O], F32R, name="wk_sb")
                nc.sync.dma_start(out=wq_sb[:], in_=wqt[:].rearrange("(dc p) o -> p dc o", p=128))
                nc.sync.dma_start(out=wk_sb[:], in_=wkt[:].rearrange("(dc p) o -> p dc o", p=128))

                for tm in range(TM):
                    pq = [ps.tile([128, O], F32, name=f"pq{i}", tag=f"pq{i}") for i in range(4)]
                    pk = [ps.tile([128, O], F32, name=f"pk{i}", tag=f"pk{i}") for i in range(4)]
                    for d in range(DC):
                        x_t = sb.tile([128, 512], F32R, name="x_t", tag="x_t", bufs=4)
                        nc.sync.dma_start(
                            out=x_t[:], in_=xt[d * 128:(d + 1) * 128, tm * 512:(tm + 1) * 512])
                        for ts in range(4):
                            lhs = x_t[:, ts * 128:(ts + 1) * 128]
                            nc.tensor.matmul(pq[ts][:], lhs, wq_sb[:, d, :],
                                             start=(d == 0), stop=(d == DC - 1))
                            nc.tensor.matmul(pk[ts][:], lhs, wk_sb[:, d, :],
                                             start=(d == 0), stop=(d == DC - 1))
                    for ts in range(4):
                        tt_i = tm * 4 + ts
                        cosb = cos_sb[:, tt_i, :].unsqueeze(1).broadcast_to([128, H_PER, 64])
                        sinb = sin_sb[:, tt_i, :].unsqueeze(1).broadcast_to([128, H_PER, 64])
                        for which, psrc, dst in (("q", pq[ts], q_s), ("k", pk[ts], k_s)):
                            rot = sb.tile([128, O], F32R, name=f"rot{which}",
                                          tag=f"rot{which}", bufs=3)
                            tmp = sb.tile([128, H_PER, 64], F32, name=f"tmp{which}",
                                          tag=f"tmp{which}", bufs=3)
                            pv = psrc[:].rearrange("p (h e k) -> p h e k", e=2, k=64)
                            rv = rot[:].rearrange("p (h e k) -> p h e k", e=2, k=64)
                            pe, po = pv[:, :, 0, :], pv[:, :, 1, :]
                            re, ro = rv[:, :, 0, :], rv[:, :, 1, :]
                            nc.vector.tensor_tensor(re, pe, cosb, ALU.mult)
                            nc.vector.tensor_tensor(tmp[:], po, sinb, ALU.mult)
                            nc.vector.tensor_tensor(re, re, tmp[:], ALU.subtract)
                            nc.vector.tensor_tensor(ro, po, cosb, ALU.mult)
                            nc.vector.tensor_tensor(tmp[:], pe, sinb, ALU.mult)
                            nc.vector.tensor_tensor(ro, ro, tmp[:], ALU.add)
                            nc.sync.dma_start(
                                out=dst[tt_i * 128:(tt_i + 1) * 128, :], in_=rot[:])

            # ================= Phase 1b: V projection =======================
            with tc.tile_pool(name="wv", bufs=1) as wpool, \
                 tc.tile_pool(name="p2sb", bufs=1) as sb, \
                 tc.tile_pool(name="p2ps", bufs=1, space="PSUM") as ps:
                wv_sb = wpool.tile([128, DC, O], F32R, name="wv_sb")
                nc.sync.dma_start(out=wv_sb[:], in_=wvt[:].rearrange("(dc p) o -> p dc o", p=128))
                for tm in range(TM):
                    pv_ = [ps.tile([128, O], F32, name=f"pv{i}", tag=f"pv{i}", bufs=2)
                           for i in range(4)]
                    for d in range(DC):
                        x_t = sb.tile([128, 512], F32R, name="x_t", tag="x_t", bufs=4)
                        nc.sync.dma_start(
                            out=x_t[:], in_=xt[d * 128:(d + 1) * 128, tm * 512:(tm + 1) * 512])
                        for ts in range(4):
                            nc.tensor.matmul(pv_[ts][:], x_t[:, ts * 128:(ts + 1) * 128],
                                             wv_sb[:, d, :],
                                             start=(d == 0), stop=(d == DC - 1))
                    for ts in range(4):
                        tt_i = tm * 4 + ts
                        vout = sb.tile([128, O], F32R, name="vout", tag="vout", bufs=3)
                        nc.vector.tensor_copy(vout[:], pv_[ts][:])
                        nc.sync.dma_start(
                            out=v_s[tt_i * 128:(tt_i + 1) * 128, :], in_=vout[:])

            # ================= Phase 2: attention per head ==================
            with tc.tile_pool(name="otp", bufs=1) as otpool:
              ot_sb = [otpool.tile([128, T], F32R, name=f"ot{h}") for h in range(H_PER)]
              with tc.tile_pool(name="atsb", bufs=1) as sb, \
                   tc.tile_pool(name="atps", bufs=1, space="PSUM") as ps:
                for h in range(H_PER):
                    qt_h = sb.tile([128, T], F32R, name="qt_h", tag="qt_h")
                    kt_h = sb.tile([128, T], F32R, name="kt_h", tag="kt_h")
                    v_sb = sb.tile([128, TT, 128], F32R, name="v_sb", tag="v_sb")
                    nc.sync.dma_start(
                        out=v_sb[:],
                        in_=v_s[:, h * 128:(h + 1) * 128].rearrange("(jt p) e -> p jt e", p=128))
                    # transpose q/k [t, d] tiles -> [d, t]
                    for which, src, dst in (("q", q_s, qt_h), ("k", k_s, kt_h)):
                        for tt_i in range(TT):
                            ld = sb.tile([128, 128], F32R, name=f"ld{which}",
                                         tag=f"ld{which}", bufs=4)
                            nc.sync.dma_start(
                                out=ld[:],
                                in_=src[tt_i * 128:(tt_i + 1) * 128, h * 128:(h + 1) * 128])
                            ptr = ps.tile([128, 128], F32R, name="ptr", tag="ptr", bufs=2)
                            nc.tensor.transpose(ptr[:], ld[:], id_sb[:])
                            nc.vector.tensor_copy(dst[:, tt_i * 128:(tt_i + 1) * 128], ptr[:])

                    pt_st = sb.tile([128, cfg.NJT, 512], F32R, name="pt_st", tag="pt_st")
                    for b in range(B):
                        for im in range(cfg.IMACS):
                            i0 = b * S + im * 512
                            njt = 4 * (im + 1)
                            p_ot = ps.tile([128, 512], F32, name="p_ot", tag="p_ot", bufs=2)
                            p_s = ps.tile([1, 512], F32, name="p_s", tag="p_s", bufs=1)
                            for jt in range(njt):
                                j0 = b * S + jt * 128
                                p_st = ps.tile([128, 512], F32, name="p_st", tag="p_st", bufs=2)
                                nc.tensor.matmul(p_st[:], kt_h[:, j0:j0 + 128],
                                                 qt_h[:, i0:i0 + 512], start=True, stop=True)
                                if jt >= 4 * im:  # diagonal band: causal mask
                                    nc.vector.tensor_tensor(
                                        p_st[:], p_st[:], mask_sb[:, jt - 4 * im, :], ALU.add)
                                nc.scalar.activation(pt_st[:, jt, :], p_st[:], AF.Exp,
                                                     scale=scale)
                                nc.tensor.matmul(p_s[:], ones_sb[:, 0:1], pt_st[:, jt, :],
                                                 start=(jt == 0), stop=(jt == njt - 1))
                                nc.tensor.matmul(p_ot[:], v_sb[:, b * cfg.NJT + jt, :],
                                                 pt_st[:, jt, :],
                                                 start=(jt == 0), stop=(jt == njt - 1))
                            rcp = sb.tile([1, 512], F32R, name="rcp", tag="rcp", bufs=2)
                            with nc.allow_low_precision(reason="softmax recip in f32r"):
                                nc.vector.reciprocal(rcp[:], p_s[:])
                            p_r = ps.tile([128, 512], F32, name="p_r", tag="p_r", bufs=1)
                            nc.tensor.matmul(p_r[:], ones_sb[0:1, :], rcp[:],
                                             start=True, stop=True)
                            r_sb = sb.tile([128, 512], F32, name="r_sb", tag="r_sb", bufs=2)
                            nc.vector.tensor_copy(r_sb[:], p_r[:])
                            nc.vector.tensor_tensor(ot_sb[h][:, i0:i0 + 512], p_ot[:],
                                                    r_sb[:], ALU.mult)

              # ================= Phase 3: output projection ===============
              with tc.tile_pool(name="wop", bufs=1) as wopool, \
                   tc.tile_pool(name="p3sb", bufs=1) as sb3, \
                   tc.tile_pool(name="p3ps", bufs=1, space="PSUM") as ps3:
                    wo_sb = wopool.tile([128, H_PER, D], F32R, name="wo_sb")
                    nc.sync.dma_start(
                        out=wo_sb[:], in_=wot[:].rearrange("(hh p) e -> p hh e", p=128))
                    for tt_i in range(TT):
                        pf = [ps3.tile([128, 512], F32, name=f"pf{e}", tag=f"pf{e}")
                              for e in range(cfg.EC)]
                        for h in range(H_PER):
                            for e in range(cfg.EC):
                                nc.tensor.matmul(
                                    pf[e][:], ot_sb[h][:, tt_i * 128:(tt_i + 1) * 128],
                                    wo_sb[:, h, e * 512:(e + 1) * 512],
                                    start=(h == 0), stop=(h == H_PER - 1))
                        for e in range(cfg.EC):
                            f_sb = sb3.tile([128, 512], F32, name="f_sb", tag="f_sb", bufs=4)
                            nc.vector.tensor_copy(f_sb[:], pf[e][:])
                            nc.sync.dma_start(
                                out=out[tt_i * 128:(tt_i + 1) * 128, e * 512:(e + 1) * 512],
                                in_=f_sb[:])

    nc.compile()
    return nc


# host-side even/odd permutation of head dims (RoPE becomes half-split elementwise)
_PERM = np.concatenate([np.arange(0, HEAD_DIM, 2), np.arange(1, HEAD_DIM, 2)])


def host_inputs(cfg: Cfg, x, wq, wk, wv, wo, freqs_cos, freqs_sin):
    """Build the 8 per-core input maps from full inputs (numpy f32)."""
    B, S, D, T, O, H_PER = cfg.B, cfg.S, cfg.D, cfg.T, cfg.O, cfg.H_PER
    xt = np.ascontiguousarray(x.reshape(T, D).T)

    cose = np.ascontiguousarray(np.tile(freqs_cos, (B, 1)))  # [T, 64]
    sine = np.ascontiguousarray(np.tile(freqs_sin, (B, 1)))

    # maskt[p*128 + j, i] = 0 if p*128 + j <= i else NEG
    j_idx = np.arange(4 * 128)[:, None]
    i_idx = np.arange(512)[None, :]
    maskt = np.where(j_idx <= i_idx, 0.0, NEG).astype(np.float32)

    ident = np.eye(128, dtype=np.float32)
    onesm = np.ones((128, 128), dtype=np.float32)

    in_maps = []
    for c in range(cfg.n_cores):
        rows = []
        for hh in range(H_PER):
            base = (c * H_PER + hh) * HEAD_DIM
            rows.append(base + _PERM)
        prows = np.concatenate(rows)                     # permuted rows for q/k
        nrows = np.arange(c * O, (c + 1) * O)            # natural rows for v
        in_maps.append({
            "xt": xt,
            "wqt": np.ascontiguousarray(wq[prows].T),
            "wkt": np.ascontiguousarray(wk[prows].T),
            "wvt": np.ascontiguousarray(wv[nrows].T),
            "wot": np.ascontiguousarray(wo[:, nrows].T),
            "cose": cose, "sine": sine, "maskt": maskt,
            "ident": ident, "onesm": onesm,
        })
    return in_maps


_CACHE = {}


def kernel(x, wq, wk, wv, wo, freqs_cos, freqs_sin, mask=None, start_pos=0):
    cfg = Cfg()
    x = np.asarray(x, dtype=np.float32)
    in_maps = host_inputs(cfg, x, np.asarray(wq, np.float32), np.asarray(wk, np.float32),
                          np.asarray(wv, np.float32), np.asarray(wo, np.float32),
                          np.asarray(freqs_cos, np.float32), np.asarray(freqs_sin, np.float32))
    if "nc" not in _CACHE:
        _CACHE["nc"] = build(cfg)
    from concourse.bass_utils import run_bass_kernel_spmd
    res = run_bass_kernel_spmd(_CACHE["nc"], in_maps, core_ids=list(range(cfg.n_cores)))
    acc = res.results[0]["out"]
    for c in range(1, cfg.n_cores):
        acc = acc + res.results[c]["out"]
    return acc.reshape(cfg.B, cfg.S, cfg.D)


# revision 8
# speedup vs baseline: 1.0200x; 1.0200x over previous
"""Tensor-parallel attention kernel for 8 Trainium2 NeuronCores.

Shards the 32 attention heads across 8 cores (4 heads each): wq/wk/wv are
sharded column-wise (rows of the weight matrices), wo row-wise; x is
replicated. Each core computes a partial output (its heads' contribution
through wo) and the host sums the 8 partials.

All device matmuls run in float32r (fp32 storage, reduced-precision PE mode,
full bf16-rate for moving dims >= 256), accumulating in f32 PSUM.
"""

import math
import sys

sys.path.insert(0, "/opt/trn_rl_repo")

import numpy as np

import concourse.bacc as bacc
import concourse.bass as bass
import concourse.mybir as mybir
import concourse.tile as tile

F32 = mybir.dt.float32
F32R = mybir.dt.float32r
AF = mybir.ActivationFunctionType
ALU = mybir.AluOpType

HEAD_DIM = 128
NEG = -1.0e30


class Cfg:
    def __init__(self, B=2, S=2048, D=4096, H_PER=4, n_cores=8):
        self.B, self.S, self.D, self.H_PER = B, S, D, H_PER
        self.n_cores = n_cores
        self.T = B * S                    # total tokens (batch-major)
        self.O = H_PER * HEAD_DIM         # per-core projection width
        self.DC = D // 128                # contraction chunks
        self.TT = self.T // 128           # 128-token tiles
        self.TM = self.T // 256           # 256-token macro tiles (proj phases)
        self.NJT = S // 128               # max j-tiles per batch
        self.IMACS = S // 512             # 512-wide i-macros per batch
        self.EC = D // 512                # 512-wide e-chunks of the output


def build(cfg: Cfg) -> bacc.Bacc:
    B, S, D, T, O = cfg.B, cfg.S, cfg.D, cfg.T, cfg.O
    H_PER, DC, TT, TM = cfg.H_PER, cfg.DC, cfg.TT, cfg.TM
    scale = 1.0 / math.sqrt(HEAD_DIM)

    nc = bacc.Bacc(None, target_bir_lowering=False)

    xt = nc.dram_tensor("xt", [D, T], F32R, kind="ExternalInput")
    wqt = nc.dram_tensor("wqt", [D, O], F32R, kind="ExternalInput")
    wkt = nc.dram_tensor("wkt", [D, O], F32R, kind="ExternalInput")
    wvt = nc.dram_tensor("wvt", [D, O], F32R, kind="ExternalInput")
    wot = nc.dram_tensor("wot", [O, D], F32R, kind="ExternalInput")
    cose = nc.dram_tensor("cose", [T, 64], F32, kind="ExternalInput")
    sine = nc.dram_tensor("sine", [T, 64], F32, kind="ExternalInput")
    maskt = nc.dram_tensor("maskt", [4 * 128, 512], F32, kind="ExternalInput")
    ident = nc.dram_tensor("ident", [128, 128], F32R, kind="ExternalInput")
    onesm = nc.dram_tensor("onesm", [128, 128], F32R, kind="ExternalInput")
    out = nc.dram_tensor("out", [T, D], F32, kind="ExternalOutput")

    # DRAM scratch for projected q/k (rotated) and v, [token, O] layout
    q_s = nc.dram_tensor("q_s", [T, O], F32R)
    k_s = nc.dram_tensor("k_s", [T, O], F32R)
    v_s = nc.dram_tensor("v_s", [T, O], F32R)

    with tile.TileContext(nc) as tc:
        with tc.tile_pool(name="const", bufs=1) as cpool:
            mask_sb = cpool.tile([128, 4, 512], F32, name="mask_sb")
            id_sb = cpool.tile([128, 128], F32R, name="id_sb")
            ones_sb = cpool.tile([128, 128], F32R, name="ones_sb")
            nc.sync.dma_start(out=mask_sb[:], in_=maskt[:].rearrange("(m p) j -> p m j", p=128))
            nc.sync.dma_start(out=id_sb[:], in_=ident[:])
            nc.sync.dma_start(out=ones_sb[:], in_=onesm[:])

            # ================= Phase 1a: Q,K projections + RoPE =============
            with tc.tile_pool(name="wqk", bufs=1) as wpool, \
                 tc.tile_pool(name="p1sb", bufs=1) as sb, \
                 tc.tile_pool(name="p1ps", bufs=1, space="PSUM") as ps:
                cos_sb = wpool.tile([128, TT, 64], F32, name="cos_sb")
                sin_sb = wpool.tile([128, TT, 64], F32, name="sin_sb")
                nc.sync.dma_start(out=cos_sb[:], in_=cose[:].rearrange("(tt p) k -> p tt k", p=128))
                nc.sync.dma_start(out=sin_sb[:], in_=sine[:].rearrange("(tt p) k -> p tt k", p=128))
                wq_sb = wpool.tile([128, DC, O], F32R, name="wq_sb")
                wk_sb = wpool.tile([128, DC, O], F32R, name="wk_sb")
                for d in range(DC):
                    nc.sync.dma_start(out=wq_sb[:, d, :],
                                      in_=wqt[d * 128:(d + 1) * 128, :])
                    nc.sync.dma_start(out=wk_sb[:, d, :],
                                      in_=wkt[d * 128:(d + 1) * 128, :])

                for tm in range(TM):
                    pq = [ps.tile([128, O], F32, name=f"pq{i}", tag=f"pq{i}", bufs=2)
                          for i in range(2)]
                    pk = [ps.tile([128, O], F32, name=f"pk{i}", tag=f"pk{i}", bufs=2)
                          for i in range(2)]
                    for d in range(DC):
                        x_t = sb.tile([128, 256], F32R, name="x_t", tag="x_t", bufs=6)
                        nc.sync.dma_start(
                            out=x_t[:], in_=xt[d * 128:(d + 1) * 128, tm * 256:(tm + 1) * 256])
                        for ts in range(2):
                            lhs = x_t[:, ts * 128:(ts + 1) * 128]
                            nc.tensor.matmul(pq[ts][:], lhs, wq_sb[:, d, :],
                                             start=(d == 0), stop=(d == DC - 1))
                            nc.tensor.matmul(pk[ts][:], lhs, wk_sb[:, d, :],
                                             start=(d == 0), stop=(d == DC - 1))
                    for ts in range(2):
                        tt_i = tm * 2 + ts
                        cosb = cos_sb[:, tt_i, :].unsqueeze(1).broadcast_to([128, H_PER, 64])
                        sinb = sin_sb[:, tt_i, :].unsqueeze(1).broadcast_to([128, H_PER, 64])
                        for which, psrc, dst in (("q", pq[ts], q_s), ("k", pk[ts], k_s)):
                            rot = sb.tile([128, O], F32R, name=f"rot{which}",
                                          tag=f"rot{which}", bufs=3)
                            tmp = sb.tile([128, H_PER, 64], F32, name=f"tmp{which}",
                                          tag=f"tmp{which}", bufs=3)
                            pv = psrc[:].rearrange("p (h e k) -> p h e k", e=2, k=64)
                            rv = rot[:].rearrange("p (h e k) -> p h e k", e=2, k=64)
                            pe, po = pv[:, :, 0, :], pv[:, :, 1, :]
                            re, ro = rv[:, :, 0, :], rv[:, :, 1, :]
                            nc.vector.tensor_tensor(re, pe, cosb, ALU.mult)
                            nc.vector.tensor_tensor(tmp[:], po, sinb, ALU.mult)
                            nc.vector.tensor_tensor(re, re, tmp[:], ALU.subtract)
                            nc.vector.tensor_tensor(ro, po, cosb, ALU.mult)
                            nc.vector.tensor_tensor(tmp[:], pe, sinb, ALU.mult)
                            nc.vector.tensor_tensor(ro, ro, tmp[:], ALU.add)
                            nc.sync.dma_start(
                                out=dst[tt_i * 128:(tt_i + 1) * 128, :], in_=rot[:])

            # q/k [t,d] -> [d,t] transposes, emitted per head (h=0 overlaps V pass)
            def transpose_qk(sb, ps, h, qt_h, kt_h):
                for which, src, dstt in (("q", q_s, qt_h), ("k", k_s, kt_h)):
                    for tt_i in range(TT):
                        ld = sb.tile([128, 128], F32R, name=f"ld{which}",
                                     tag=f"ld{which}", bufs=4)
                        nc.sync.dma_start(
                            out=ld[:],
                            in_=src[tt_i * 128:(tt_i + 1) * 128, h * 128:(h + 1) * 128])
                        ptr = ps.tile([128, 128], F32R, name="ptr", tag="ptr", bufs=2)
                        nc.tensor.transpose(ptr[:], ld[:], id_sb[:])
                        nc.vector.tensor_copy(dstt[:, tt_i * 128:(tt_i + 1) * 128], ptr[:])

            with tc.tile_pool(name="otp", bufs=1) as otpool:
              ot_sb = [otpool.tile([128, T], F32R, name=f"ot{h}") for h in range(H_PER)]
              with tc.tile_pool(name="qkt", bufs=1) as qkpool, \
                 tc.tile_pool(name="trps", bufs=1, space="PSUM") as trps:
                qt_h = qkpool.tile([128, T], F32R, name="qt_h", tag="qt_h")
                kt_h = qkpool.tile([128, T], F32R, name="kt_h", tag="kt_h")
                transpose_qk(qkpool, trps, 0, qt_h, kt_h)

                # ================= Phase 1b: V projection ===================
                with tc.tile_pool(name="wv", bufs=1) as wpool, \
                     tc.tile_pool(name="p2sb", bufs=1) as sb, \
                     tc.tile_pool(name="p2ps", bufs=1, space="PSUM") as ps:
                    wv_sb = wpool.tile([128, DC, O], F32R, name="wv_sb")
                    for d in range(DC):
                        nc.sync.dma_start(out=wv_sb[:, d, :],
                                          in_=wvt[d * 128:(d + 1) * 128, :])
                    for tm in range(TM):
                        pv_ = [ps.tile([128, O], F32, name=f"pv{i}", tag=f"pv{i}", bufs=2)
                               for i in range(2)]
                        for d in range(DC):
                            x_t = sb.tile([128, 256], F32R, name="x_t", tag="x_t", bufs=6)
                            nc.sync.dma_start(
                                out=x_t[:],
                                in_=xt[d * 128:(d + 1) * 128, tm * 256:(tm + 1) * 256])
                            for ts in range(2):
                                nc.tensor.matmul(pv_[ts][:], x_t[:, ts * 128:(ts + 1) * 128],
                                                 wv_sb[:, d, :],
                                                 start=(d == 0), stop=(d == DC - 1))
                        for ts in range(2):
                            tt_i = tm * 2 + ts
                            vout = sb.tile([128, O], F32R, name="vout", tag="vout", bufs=3)
                            nc.vector.tensor_copy(vout[:], pv_[ts][:])
                            nc.sync.dma_start(
                                out=v_s[tt_i * 128:(tt_i + 1) * 128, :], in_=vout[:])

                # ================= Phase 2: attention per head ==============
                with tc.tile_pool(name="atsb", bufs=1) as sb, \
                     tc.tile_pool(name="atps", bufs=1, space="PSUM") as ps:
                    for h in range(H_PER):
                        if h > 0:
                            qt_h = qkpool.tile([128, T], F32R, name="qt_h", tag="qt_h")
                            kt_h = qkpool.tile([128, T], F32R, name="kt_h", tag="kt_h")
                            transpose_qk(qkpool, trps, h, qt_h, kt_h)
                        v_sb = sb.tile([128, TT, 128], F32R, name="v_sb", tag="v_sb")
                        nc.sync.dma_start(
                            out=v_sb[:],
                            in_=v_s[:, h * 128:(h + 1) * 128].rearrange(
                                "(jt p) e -> p jt e", p=128))

                        pt_st = sb.tile([128, cfg.NJT, 512], F32R, name="pt_st", tag="pt_st")
                        for b in range(B):
                            for im in range(cfg.IMACS):
                                i0 = b * S + im * 512
                                njt = 4 * (im + 1)
                                p_ot = ps.tile([128, 512], F32, name="p_ot", tag="p_ot", bufs=2)
                                p_s = ps.tile([1, 512], F32, name="p_s", tag="p_s", bufs=1)
                                for jt in range(njt):
                                    j0 = b * S + jt * 128
                                    p_st = ps.tile([128, 512], F32, name="p_st",
                                                   tag="p_st", bufs=2)
                                    nc.tensor.matmul(p_st[:], kt_h[:, j0:j0 + 128],
                                                     qt_h[:, i0:i0 + 512],
                                                     start=True, stop=True)
                                    if jt >= 4 * im:  # diagonal band: causal mask
                                        nc.vector.tensor_tensor(
                                            p_st[:], p_st[:],
                                            mask_sb[:, jt - 4 * im, :], ALU.add)
                                    nc.scalar.activation(pt_st[:, jt, :], p_st[:], AF.Exp,
                                                         scale=scale)
                                    nc.tensor.matmul(p_s[:], ones_sb[:, 0:1], pt_st[:, jt, :],
                                                     start=(jt == 0), stop=(jt == njt - 1))
                                    nc.tensor.matmul(p_ot[:], v_sb[:, b * cfg.NJT + jt, :],
                                                     pt_st[:, jt, :],
                                                     start=(jt == 0), stop=(jt == njt - 1))
                                rcp = sb.tile([1, 512], F32R, name="rcp", tag="rcp", bufs=2)
                                with nc.allow_low_precision(reason="softmax recip in f32r"):
                                    nc.vector.reciprocal(rcp[:], p_s[:])
                                p_r = ps.tile([128, 512], F32, name="p_r", tag="p_r", bufs=1)
                                nc.tensor.matmul(p_r[:], ones_sb[0:1, :], rcp[:],
                                                 start=True, stop=True)
                                r_sb = sb.tile([128, 512], F32, name="r_sb", tag="r_sb", bufs=2)
                                nc.vector.tensor_copy(r_sb[:], p_r[:])
                                nc.vector.tensor_tensor(ot_sb[h][:, i0:i0 + 512], p_ot[:],
                                                        r_sb[:], ALU.mult)

              # ============= Phase 3: output projection ===============
              if True:
                    with tc.tile_pool(name="wop", bufs=1) as wopool, \
                         tc.tile_pool(name="p3sb", bufs=1) as sb3, \
                         tc.tile_pool(name="p3ps", bufs=1, space="PSUM") as ps3:
                        wo_sb = wopool.tile([128, H_PER, D], F32R, name="wo_sb")
                        for hh in range(H_PER):
                            nc.sync.dma_start(out=wo_sb[:, hh, :],
                                              in_=wot[hh * 128:(hh + 1) * 128, :])
                        for tt_i in range(TT):
                            pf = [ps3.tile([128, 512], F32, name=f"pf{e}", tag=f"pf{e}")
                                  for e in range(cfg.EC)]
                            for h in range(H_PER):
                                for e in range(cfg.EC):
                                    nc.tensor.matmul(
                                        pf[e][:], ot_sb[h][:, tt_i * 128:(tt_i + 1) * 128],
                                        wo_sb[:, h, e * 512:(e + 1) * 512],
                                        start=(h == 0), stop=(h == H_PER - 1))
                            for e in range(cfg.EC):
                                f_sb = sb3.tile([128, 512], F32, name="f_sb",
                                                tag="f_sb", bufs=4)
                                nc.vector.tensor_copy(f_sb[:], pf[e][:])
                                nc.sync.dma_start(
                                    out=out[tt_i * 128:(tt_i + 1) * 128,
                                            e * 512:(e + 1) * 512],
                                    in_=f_sb[:])

    nc.compile()
    return nc


# host-side even/odd permutation of head dims (RoPE becomes half-split elementwise)
_PERM = np.concatenate([np.arange(0, HEAD_DIM, 2), np.arange(1, HEAD_DIM, 2)])


def host_inputs(cfg: Cfg, x, wq, wk, wv, wo, freqs_cos, freqs_sin):
    """Build the 8 per-core input maps from full inputs (numpy f32)."""
    B, S, D, T, O, H_PER = cfg.B, cfg.S, cfg.D, cfg.T, cfg.O, cfg.H_PER
    xt = np.ascontiguousarray(x.reshape(T, D).T)

    cose = np.ascontiguousarray(np.tile(freqs_cos, (B, 1)))  # [T, 64]
    sine = np.ascontiguousarray(np.tile(freqs_sin, (B, 1)))

    # maskt[p*128 + j, i] = 0 if p*128 + j <= i else NEG
    j_idx = np.arange(4 * 128)[:, None]
    i_idx = np.arange(512)[None, :]
    maskt = np.where(j_idx <= i_idx, 0.0, NEG).astype(np.float32)

    ident = np.eye(128, dtype=np.float32)
    onesm = np.ones((128, 128), dtype=np.float32)

    in_maps = []
    for c in range(cfg.n_cores):
        rows = []
        for hh in range(H_PER):
            base = (c * H_PER + hh) * HEAD_DIM
            rows.append(base + _PERM)
        prows = np.concatenate(rows)                     # permuted rows for q/k
        nrows = np.arange(c * O, (c + 1) * O)            # natural rows for v
        in_maps.append({
            "xt": xt,
            "wqt": np.ascontiguousarray(wq[prows].T),
            "wkt": np.ascontiguousarray(wk[prows].T),
            "wvt": np.ascontiguousarray(wv[nrows].T),
            "wot": np.ascontiguousarray(wo[:, nrows].T),
            "cose": cose, "sine": sine, "maskt": maskt,
            "ident": ident, "onesm": onesm,
        })
    return in_maps


_CACHE = {}


def kernel(x, wq, wk, wv, wo, freqs_cos, freqs_sin, mask=None, start_pos=0):
    cfg = Cfg()
    x = np.asarray(x, dtype=np.float32)
    in_maps = host_inputs(cfg, x, np.asarray(wq, np.float32), np.asarray(wk, np.float32),
                          np.asarray(wv, np.float32), np.asarray(wo, np.float32),
                          np.asarray(freqs_cos, np.float32), np.asarray(freqs_sin, np.float32))
    if "nc" not in _CACHE:
        _CACHE["nc"] = build(cfg)
    from concourse.bass_utils import run_bass_kernel_spmd
    res = run_bass_kernel_spmd(_CACHE["nc"], in_maps, core_ids=list(range(cfg.n_cores)))
    acc = res.results[0]["out"]
    for c in range(1, cfg.n_cores):
        acc = acc + res.results[c]["out"]
    return acc.reshape(cfg.B, cfg.S, cfg.D)
